# revision 7
# baseline (speedup 1.0000x reference)
"""Trainium2 Bass kernel for the CustomAutoencoder problem.

7-layer MLP autoencoder over x[8192, 4096], data-parallel over the batch
axis across 8 NeuronCores (1024 rows/core), weights replicated.

Per-core dataflow (activations kept transposed: features on partitions,
batch on the free axis), bf16 matmul operands with fp32 PSUM accumulation:

  x[1024,4096] --cast-DMA+PE-transpose--> xT
  L1: h1T = relu(M1.T @ xT + b1)      M1 = W1*C1   [196,  B]
  L2: h2T = relu(M2.T @ h1T + b2)     M2 = W2*C2   [10,   B]
  L3: h3T = relu(W3.T @ h2T + b3)                  [1024, B]
  L4: zT  = relu(W4.T @ h3T + b4)                  [32,   B]
  L5: d1T = relu(Wd1.T @ zT + bd1)                 [1024, B]
  L6: d2T = relu(Wd2.T @ d1T + bd2)                [2048, B]
  L7: out = sigmoid(d2 @ Wd3 + bd3)   (operands flipped: d2T chunks are
      lhsT, Wd3 is the moving operand -> natural [B, 4096] output, no
      exit transpose; bd3 enters PSUM via a K=1 ones-row matmul)
"""

import sys

if "/opt/trn_rl_repo" not in sys.path:
    sys.path.insert(0, "/opt/trn_rl_repo")

import numpy as np

B_FULL, S, H1, H2, D4, LAT, DD1, DD2 = 8192, 4096, 196, 10, 1024, 32, 1024, 2048
N_CORES = 8
B = B_FULL // N_CORES          # 1024 rows per core
P = 128                        # partitions
NB = B // P                    # 8 batch chunks per core
NT = 512                       # matmul free-dim tile (one PSUM bank of fp32)

_NC_CACHE = {}
TRACE = False  # set by test.py to capture an NTFF profile of the run


def build_nc():
    import concourse.bacc as bacc
    import concourse.mybir as mybir
    import concourse.tile as tile
    from concourse.masks import make_identity

    f32 = mybir.dt.float32
    bf16 = mybir.dt.bfloat16
    AF = mybir.ActivationFunctionType

    nc = bacc.Bacc("TRN2", target_bir_lowering=False, debug=False,
                   num_devices=N_CORES)

    # ---- DRAM I/O (names match setup_inputs keys; x/out are per-core shards)
    x_d = nc.dram_tensor("x", [B, S], f32, kind="ExternalInput")
    C1_d = nc.dram_tensor("C1", [S, H1], f32, kind="ExternalInput")
    W1_d = nc.dram_tensor("W1", [S, H1], f32, kind="ExternalInput")
    b1_d = nc.dram_tensor("b1", [H1], f32, kind="ExternalInput")
    C2_d = nc.dram_tensor("C2", [H1, H2], f32, kind="ExternalInput")
    W2_d = nc.dram_tensor("W2", [H1, H2], f32, kind="ExternalInput")
    b2_d = nc.dram_tensor("b2", [H2], f32, kind="ExternalInput")
    W3_d = nc.dram_tensor("W3", [H2, D4], f32, kind="ExternalInput")
    b3_d = nc.dram_tensor("b3", [D4], f32, kind="ExternalInput")
    W4_d = nc.dram_tensor("W4", [D4, LAT], f32, kind="ExternalInput")
    b4_d = nc.dram_tensor("b4", [LAT], f32, kind="ExternalInput")
    Wd1_d = nc.dram_tensor("Wd1", [LAT, DD1], f32, kind="ExternalInput")
    bd1_d = nc.dram_tensor("bd1", [DD1], f32, kind="ExternalInput")
    Wd2_d = nc.dram_tensor("Wd2", [DD1, DD2], f32, kind="ExternalInput")
    bd2_d = nc.dram_tensor("bd2", [DD2], f32, kind="ExternalInput")
    Wd3_d = nc.dram_tensor("Wd3", [DD2, S], f32, kind="ExternalInput")
    bd3_d = nc.dram_tensor("bd3", [S], f32, kind="ExternalInput")
    out_d = nc.dram_tensor("out", [B, S], f32, kind="ExternalOutput")

    NK1 = S // P   # 32 K-chunks for layer 1
    NK7 = DD2 // P  # 16 K-chunks for layer 7

    with tile.TileContext(nc) as tc:
        with (
            tc.tile_pool(name="const", bufs=1) as cpool,
            tc.tile_pool(name="acts01", bufs=1) as a0pool,
            tc.tile_pool(name="outp", bufs=4) as opool,
            tc.tile_pool(name="psum_mm", bufs=4, space="PSUM") as pmm,
            tc.tile_pool(name="psum_tr", bufs=2, space="PSUM") as ptr,
        ):
            # ---------------- constants / small weights ----------------
            ident = cpool.tile([P, P], bf16)
            make_identity(nc, ident)
            ones_row = cpool.tile([1, P], bf16)
            nc.gpsimd.memset(ones_row[:], 1.0)

            # biases as [P, chunks] fp32 (feature index = chunk*128 + p)
            b1_sb = cpool.tile([P, 2], f32)
            nc.sync.dma_start(b1_sb[:, 0:1], b1_d[0:P])
            nc.sync.dma_start(b1_sb[0 : H1 - P, 1:2], b1_d[P:H1])
            b2_sb = cpool.tile([H2, 1], f32)
            nc.sync.dma_start(b2_sb[:, 0:1], b2_d[:])
            b3_sb = cpool.tile([P, D4 // P], f32)
            nc.sync.dma_start(b3_sb[:], b3_d[:].rearrange("(o p) -> p o", p=P))
            b4_sb = cpool.tile([LAT, 1], f32)
            nc.sync.dma_start(b4_sb[:, 0:1], b4_d[:])
            bd1_sb = cpool.tile([P, DD1 // P], f32)
            nc.sync.dma_start(bd1_sb[:], bd1_d[:].rearrange("(o p) -> p o", p=P))
            bd2_sb = cpool.tile([P, DD2 // P], f32)
            nc.sync.dma_start(bd2_sb[:], bd2_d[:].rearrange("(o p) -> p o", p=P))
            bd3_row = cpool.tile([1, S], bf16)
            nc.gpsimd.dma_start(bd3_row[0:1, :], bd3_d[:])

            # masked layer-2 weights: M2 = W2*C2, K padded to 2x128 with zeros
            m2 = cpool.tile([P, 2, H2], bf16)
            nc.gpsimd.memset(m2[:], 0.0)
            w2_t = cpool.tile([P, 2, H2], bf16)
            nc.gpsimd.memset(w2_t[:], 0.0)
            nc.gpsimd.dma_start(w2_t[:, 0, :], W2_d[0:P, :])
            nc.gpsimd.dma_start(w2_t[0 : H1 - P, 1, :], W2_d[P:H1, :])
            c2_t = cpool.tile([P, 2, H2], bf16)
            nc.gpsimd.memset(c2_t[:], 0.0)
            nc.gpsimd.dma_start(c2_t[:, 0, :], C2_d[0:P, :])
            nc.gpsimd.dma_start(c2_t[0 : H1 - P, 1, :], C2_d[P:H1, :])
            nc.vector.tensor_mul(m2[:], w2_t[:], c2_t[:])

            # small persistent activations
            h1T = a0pool.tile([P, 2, B], bf16)
            nc.gpsimd.memset(h1T[:], 0.0)  # K-pad rows of chunk 1 stay zero
            h2T = a0pool.tile([P, B], bf16)
            nc.gpsimd.memset(h2T[:], 0.0)

            # mid-chain weights: pool opened early (space is reserved), but
            # the DMAs are emitted after the x loads so the x stream wins
            # the SWDGE queue.
            with tc.tile_pool(name="wts2", bufs=1) as wpool2:
                w3_sb = wpool2.tile([P, D4], bf16)
                w4_sb = wpool2.tile([P, D4 // P, LAT], bf16)
                wd1_sb = wpool2.tile([P, DD1], bf16)
                wd2_sb = wpool2.tile([P, DD1 // P, DD2], bf16)

                # ---------------- stage 1: x transpose + layer 1 ---------
                with tc.tile_pool(name="stage1", bufs=1) as spool:
                    w1_t = spool.tile([P, NK1, H1], f32)
                    nc.sync.dma_start(
                        w1_t[:], W1_d[:].rearrange("(ko p) m -> p ko m", p=P)
                    )
                    c1_t = spool.tile([P, NK1, H1], f32)
                    nc.sync.dma_start(
                        c1_t[:], C1_d[:].rearrange("(ko p) m -> p ko m", p=P)
                    )
                    m1 = spool.tile([P, NK1, H1], bf16)
                    nc.vector.tensor_mul(m1[:], w1_t[:], c1_t[:])

                    with tc.tile_pool(name="xbuf", bufs=1) as xpool:
                        xT = xpool.tile([P, NK1, B], bf16)
                        for b in range(NB):
                            for q in range(4):  # quarters of the 4096 row
                                x_nat = xpool.tile([P, 1024], bf16,
                                                   tag="xnat", bufs=3)
                                nc.gpsimd.dma_start(
                                    x_nat[:],
                                    x_d[b * P : (b + 1) * P,
                                        q * 1024 : (q + 1) * 1024],
                                )
                                for h in range(2):  # 4 transposes / psum tile
                                    pt = ptr.tile([P, 512], bf16, tag="tr")
                                    for j in range(4):
                                        nc.tensor.transpose(
                                            pt[:, j * P : (j + 1) * P],
                                            x_nat[:, (h * 4 + j) * P :
                                                  (h * 4 + j + 1) * P],
                                            ident,
                                        )
                                    dst = xT[:, q * 8 + h * 4 :
                                             q * 8 + h * 4 + 4,
                                             b * P : (b + 1) * P]
                                    src = pt[:].rearrange(
                                        "p (j c) -> p j c", c=P)
                                    if (b * 8 + q * 2 + h) % 2 == 0:
                                        nc.vector.tensor_copy(dst, src)
                                    else:
                                        nc.scalar.copy(dst, src)
                            # layer 1 for the n-th 512-wide batch tile once
                            # its 4 batch chunks are transposed
                            if b % 4 == 3:
                                n = b // 4
                                ns = slice(n * NT, (n + 1) * NT)
                                for m in range(2):
                                    mw = P if m == 0 else H1 - P
                                    ps = pmm.tile([P, NT], f32, tag="mm")
                                    for k in range(NK1):
                                        nc.tensor.matmul(
                                            ps[0:mw, :],
                                            m1[:, k, m * P : m * P + mw],
                                            xT[:, k, ns],
                                            start=(k == 0),
                                            stop=(k == NK1 - 1),
                                        )
                                    nc.scalar.activation(
                                        h1T[0:mw, m, ns], ps[0:mw, :],
                                        AF.Relu, bias=b1_sb[0:mw, m : m + 1],
                                    )
                        # mid-chain weight loads: emitted after the x loads
                        nc.gpsimd.memset(w3_sb[:], 0.0)
                        nc.gpsimd.dma_start(w3_sb[0:H2, :], W3_d[:])
                        nc.gpsimd.dma_start(
                            w4_sb[:],
                            W4_d[:].rearrange("(ko p) m -> p ko m", p=P),
                        )
                        nc.gpsimd.memset(wd1_sb[:], 0.0)
                        nc.gpsimd.dma_start(wd1_sb[0:LAT, :], Wd1_d[:])
                        nc.gpsimd.dma_start(
                            wd2_sb[:],
                            Wd2_d[:].rearrange("(ko p) m -> p ko m", p=P),
                        )

                # ------------- layers 2-6 (transposed chain) -------------
                with tc.tile_pool(name="acts2", bufs=1) as a2pool:
                    h3T = a2pool.tile([P, D4 // P, B], bf16)
                    zT = a2pool.tile([P, B], bf16)
                    nc.gpsimd.memset(zT[:], 0.0)
                    d1T = a2pool.tile([P, DD1 // P, B], bf16)
                    d2T = a2pool.tile([P, DD2 // P, B], bf16)

                    for n in range(B // NT):
                        ns = slice(n * NT, (n + 1) * NT)
                        # L2: K = 196 (2 padded chunks), M = 10
                        ps = pmm.tile([P, NT], f32, tag="mm")
                        for k in range(2):
                            nc.tensor.matmul(ps[0:H2, :], m2[:, k, :],
                                             h1T[:, k, ns],
                                             start=(k == 0), stop=(k == 1))
                        nc.scalar.activation(h2T[0:H2, ns], ps[0:H2, :],
                                             AF.Relu, bias=b2_sb[:, 0:1])
                        # L3: K = 10 (padded to 128), M = 1024
                        for m in range(D4 // P):
                            ps = pmm.tile([P, NT], f32, tag="mm")
                            nc.tensor.matmul(ps[:],
                                             w3_sb[:, m * P : (m + 1) * P],
                                             h2T[:, ns], start=True,
                                             stop=True)
                            nc.scalar.activation(h3T[:, m, ns], ps[:],
                                                 AF.Relu,
                                                 bias=b3_sb[:, m : m + 1])
                        # L4: K = 1024, M = 32
                        ps = pmm.tile([P, NT], f32, tag="mm")
                        for k in range(D4 // P):
                            nc.tensor.matmul(ps[0:LAT, :], w4_sb[:, k, :],
                                             h3T[:, k, ns], start=(k == 0),
                                             stop=(k == D4 // P - 1))
                        nc.scalar.activation(zT[0:LAT, ns], ps[0:LAT, :],
                                             AF.Relu, bias=b4_sb[:, 0:1])
                        # L5: K = 32 (padded to 128), M = 1024
                        for m in range(DD1 // P):
                            ps = pmm.tile([P, NT], f32, tag="mm")
                            nc.tensor.matmul(ps[:],
                                             wd1_sb[:, m * P : (m + 1) * P],
                                             zT[:, ns], start=True, stop=True)
                            nc.scalar.activation(d1T[:, m, ns], ps[:],
                                                 AF.Relu,
                                                 bias=bd1_sb[:, m : m + 1])
                        # L6: K = 1024, M = 2048
                        for m in range(DD2 // P):
                            ps = pmm.tile([P, NT], f32, tag="mm")
                            for k in range(DD1 // P):
                                nc.tensor.matmul(
                                    ps[:],
                                    wd2_sb[:, k, m * P : (m + 1) * P],
                                    d1T[:, k, ns], start=(k == 0),
                                    stop=(k == DD1 // P - 1),
                                )
                            nc.scalar.activation(d2T[:, m, ns], ps[:],
                                                 AF.Relu,
                                                 bias=bd2_sb[:, m : m + 1])

                    # ---------- layer 7 (flipped, natural output) ---------
                    wd3_r = Wd3_d[:].rearrange("(ko p) n -> p ko n", p=P)
                    with tc.tile_pool(name="wd3", bufs=3) as wpool3:
                        for nn in range(S // NT):
                            wt = wpool3.tile([P, NK7, NT], bf16, tag="wd3")
                            nc.gpsimd.dma_start(
                                wt[:], wd3_r[:, :, nn * NT : (nn + 1) * NT]
                            )
                            for m in range(NB):
                                ps = pmm.tile([P, NT], f32, tag="mm")
                                # bias first: K=1 ones-row outer product
                                # broadcasts bd3 to all 128 batch partitions
                                nc.tensor.matmul(
                                    ps[:], ones_row[0:1, :],
                                    bd3_row[0:1, nn * NT : (nn + 1) * NT],
                                    start=True, stop=False,
                                )
                                for k in range(NK7):
                                    nc.tensor.matmul(
                                        ps[:],
                                        d2T[:, k, m * P : (m + 1) * P],
                                        wt[:, k, :], start=False,
                                        stop=(k == NK7 - 1),
                                    )
                                ot = opool.tile([P, NT], f32, tag="out")
                                nc.scalar.activation(ot[:], ps[:], AF.Sigmoid)
                                nc.sync.dma_start(
                                    out_d[m * P : (m + 1) * P,
                                          nn * NT : (nn + 1) * NT],
                                    ot[:],
                                )

    nc.compile()
    return nc


def _get_nc():
    if "nc" not in _NC_CACHE:
        _NC_CACHE["nc"] = build_nc()
    return _NC_CACHE["nc"]


def kernel(**inputs):
    from concourse.bass_utils import run_bass_kernel_spmd

    nc = _get_nc()
    full = {k: np.ascontiguousarray(np.asarray(v, dtype=np.float32))
            for k, v in inputs.items()}
    x = full.pop("x")
    in_maps = []
    for c in range(N_CORES):
        m = dict(full)
        m["x"] = np.ascontiguousarray(x[c * B : (c + 1) * B])
        in_maps.append(m)
    res = run_bass_kernel_spmd(nc, in_maps, core_ids=list(range(N_CORES)),
                               trace=TRACE)
    _NC_CACHE["last_res"] = res
    out = np.concatenate([res.results[c]["out"] for c in range(N_CORES)],
                         axis=0)
    return out


# revision 10
# speedup vs baseline: 1.0318x; 1.0318x over previous
"""Trainium2 Bass kernel for the CustomAutoencoder problem.

7-layer MLP autoencoder over x[8192, 4096], data-parallel over the batch
axis across 8 NeuronCores (1024 rows/core), weights replicated.

Per-core dataflow (activations kept transposed: features on partitions,
batch on the free axis), bf16 matmul operands with fp32 PSUM accumulation:

  x[1024,4096] --cast-DMA+PE-transpose--> xT
  L1: h1T = relu(M1.T @ xT + b1)      M1 = W1*C1   [196,  B]
  L2: h2T = relu(M2.T @ h1T + b2)     M2 = W2*C2   [10,   B]
  L3: h3T = relu(W3.T @ h2T + b3)                  [1024, B]
  L4: zT  = relu(W4.T @ h3T + b4)                  [32,   B]
  L5: d1T = relu(Wd1.T @ zT + bd1)                 [1024, B]
  L6: d2T = relu(Wd2.T @ d1T + bd2)                [2048, B]
  L7: out = sigmoid(d2 @ Wd3 + bd3)   (operands flipped: d2T chunks are
      lhsT, Wd3 is the moving operand -> natural [B, 4096] output, no
      exit transpose; bd3 enters PSUM via a K=1 ones-row matmul)
"""

import sys

if "/opt/trn_rl_repo" not in sys.path:
    sys.path.insert(0, "/opt/trn_rl_repo")

import numpy as np

B_FULL, S, H1, H2, D4, LAT, DD1, DD2 = 8192, 4096, 196, 10, 1024, 32, 1024, 2048
N_CORES = 8
B = B_FULL // N_CORES          # 1024 rows per core
P = 128                        # partitions
NB = B // P                    # 8 batch chunks per core
NT = 512                       # matmul free-dim tile (one PSUM bank of fp32)

_NC_CACHE = {}
TRACE = False  # set by test.py to capture an NTFF profile of the run


def build_nc():
    import concourse.bacc as bacc
    import concourse.mybir as mybir
    import concourse.tile as tile
    from concourse.masks import make_identity

    f32 = mybir.dt.float32
    bf16 = mybir.dt.bfloat16
    AF = mybir.ActivationFunctionType

    nc = bacc.Bacc("TRN2", target_bir_lowering=False, debug=False,
                   num_devices=N_CORES)

    # ---- DRAM I/O (names match setup_inputs keys; x/out are per-core shards)
    x_d = nc.dram_tensor("x", [B, S], f32, kind="ExternalInput")
    C1_d = nc.dram_tensor("C1", [S, H1], f32, kind="ExternalInput")
    W1_d = nc.dram_tensor("W1", [S, H1], f32, kind="ExternalInput")
    b1_d = nc.dram_tensor("b1", [H1], f32, kind="ExternalInput")
    C2_d = nc.dram_tensor("C2", [H1, H2], f32, kind="ExternalInput")
    W2_d = nc.dram_tensor("W2", [H1, H2], f32, kind="ExternalInput")
    b2_d = nc.dram_tensor("b2", [H2], f32, kind="ExternalInput")
    W3_d = nc.dram_tensor("W3", [H2, D4], f32, kind="ExternalInput")
    b3_d = nc.dram_tensor("b3", [D4], f32, kind="ExternalInput")
    W4_d = nc.dram_tensor("W4", [D4, LAT], f32, kind="ExternalInput")
    b4_d = nc.dram_tensor("b4", [LAT], f32, kind="ExternalInput")
    Wd1_d = nc.dram_tensor("Wd1", [LAT, DD1], f32, kind="ExternalInput")
    bd1_d = nc.dram_tensor("bd1", [DD1], f32, kind="ExternalInput")
    Wd2_d = nc.dram_tensor("Wd2", [DD1, DD2], f32, kind="ExternalInput")
    bd2_d = nc.dram_tensor("bd2", [DD2], f32, kind="ExternalInput")
    Wd3_d = nc.dram_tensor("Wd3", [DD2, S], f32, kind="ExternalInput")
    bd3_d = nc.dram_tensor("bd3", [S], f32, kind="ExternalInput")
    out_d = nc.dram_tensor("out", [B, S], f32, kind="ExternalOutput")

    NK1 = S // P   # 32 K-chunks for layer 1
    NK7 = DD2 // P  # 16 K-chunks for layer 7

    with tile.TileContext(nc) as tc:
        with (
            tc.tile_pool(name="const", bufs=1) as cpool,
            tc.tile_pool(name="acts01", bufs=1) as a0pool,
            tc.tile_pool(name="outp", bufs=4) as opool,
        ):
            # ---------------- constants / small weights ----------------
            ident = cpool.tile([P, P], bf16)
            make_identity(nc, ident)
            ones_row = cpool.tile([1, P], bf16)
            nc.gpsimd.memset(ones_row[:], 1.0)

            # biases as [P, chunks] fp32 (feature index = chunk*128 + p)
            b1_sb = cpool.tile([P, 2], f32)
            nc.sync.dma_start(b1_sb[:, 0:1], b1_d[0:P])
            nc.sync.dma_start(b1_sb[0 : H1 - P, 1:2], b1_d[P:H1])
            b2_sb = cpool.tile([H2, 1], f32)
            nc.sync.dma_start(b2_sb[:, 0:1], b2_d[:])
            b3_sb = cpool.tile([P, D4 // P], f32)
            nc.sync.dma_start(b3_sb[:], b3_d[:].rearrange("(o p) -> p o", p=P))
            b4_sb = cpool.tile([LAT, 1], f32)
            nc.sync.dma_start(b4_sb[:, 0:1], b4_d[:])
            bd1_sb = cpool.tile([P, DD1 // P], f32)
            nc.sync.dma_start(bd1_sb[:], bd1_d[:].rearrange("(o p) -> p o", p=P))
            bd2_sb = cpool.tile([P, DD2 // P], f32)
            nc.sync.dma_start(bd2_sb[:], bd2_d[:].rearrange("(o p) -> p o", p=P))
            bd3_row = cpool.tile([1, S], bf16)
            nc.gpsimd.dma_start(bd3_row[0:1, :], bd3_d[:])

            # masked layer-2 weights: M2 = W2*C2, K padded to 2x128 with zeros
            m2 = cpool.tile([P, 2, H2], bf16)
            nc.gpsimd.memset(m2[:], 0.0)
            w2_t = cpool.tile([P, 2, H2], bf16)
            nc.gpsimd.memset(w2_t[:], 0.0)
            nc.gpsimd.dma_start(w2_t[:, 0, :], W2_d[0:P, :])
            nc.gpsimd.dma_start(w2_t[0 : H1 - P, 1, :], W2_d[P:H1, :])
            c2_t = cpool.tile([P, 2, H2], bf16)
            nc.gpsimd.memset(c2_t[:], 0.0)
            nc.gpsimd.dma_start(c2_t[:, 0, :], C2_d[0:P, :])
            nc.gpsimd.dma_start(c2_t[0 : H1 - P, 1, :], C2_d[P:H1, :])
            nc.vector.tensor_mul(m2[:], w2_t[:], c2_t[:])

            # small persistent activations
            h1T = a0pool.tile([P, 2, B], bf16)
            nc.gpsimd.memset(h1T[:], 0.0)  # K-pad rows of chunk 1 stay zero
            h2T = a0pool.tile([P, B], bf16)
            nc.gpsimd.memset(h2T[:], 0.0)

            # mid-chain weights: pool opened early (space is reserved), but
            # the DMAs are emitted after the x loads so the x stream wins
            # the SWDGE queue.
            with tc.tile_pool(name="wts2", bufs=1) as wpool2:
                w3_sb = wpool2.tile([P, D4], bf16)
                w4_sb = wpool2.tile([P, D4 // P, LAT], bf16)
                wd1_sb = wpool2.tile([P, DD1], bf16)
                wd2_sb = wpool2.tile([P, DD1 // P, DD2], bf16)

                # ---------------- stage 1: x transpose + layer 1 ---------
                with (
                    tc.tile_pool(name="stage1", bufs=1) as spool,
                    tc.tile_pool(name="psum_tr", bufs=2,
                                 space="PSUM") as ptr,
                    tc.tile_pool(name="psum_l1", bufs=1,
                                 space="PSUM") as pl1,
                ):
                    # PE warm-up: ~40 back-to-back matmuls lift the HAM
                    # clock gate (1.2 -> 2.4 GHz) before the real work.
                    warm_ps = ptr.tile([P, P], f32, tag="warm", bufs=1)
                    for _ in range(40):
                        nc.tensor.matmul(warm_ps[:], ident[:], ident[:],
                                         start=True, stop=True,
                                         skip_group_check=True)

                    w1_t = spool.tile([P, NK1, H1], f32)
                    nc.sync.dma_start(
                        w1_t[:], W1_d[:].rearrange("(ko p) m -> p ko m", p=P)
                    )
                    c1_t = spool.tile([P, NK1, H1], f32)
                    nc.sync.dma_start(
                        c1_t[:], C1_d[:].rearrange("(ko p) m -> p ko m", p=P)
                    )
                    m1 = spool.tile([P, NK1, H1], bf16)
                    nc.vector.tensor_mul(m1[:], w1_t[:], c1_t[:])

                    # 4 PSUM accumulators, each holding two 256-wide batch
                    # regions. start=False throughout (a start=True would
                    # clear has_written for the whole bank, wiping the
                    # sibling region), so zero them explicitly first.
                    ps_l1 = [[pl1.tile([P, NT], f32, name=f"l1_{m}_{j}")
                              for j in range(2)] for m in range(2)]
                    for m in range(2):
                        for j in range(2):
                            nc.any.memset(ps_l1[m][j][:], 0.0)

                    with tc.tile_pool(name="xbuf", bufs=1) as xpool:
                        xT = xpool.tile([P, NK1, B], bf16)
                        for q in range(4):      # quarters of the 4096 row
                            for b in range(NB):  # batch chunks of 128
                                x_nat = xpool.tile([P, 1024], bf16,
                                                   tag="xnat", bufs=3)
                                nc.gpsimd.dma_start(
                                    x_nat[:],
                                    x_d[b * P : (b + 1) * P,
                                        q * 1024 : (q + 1) * 1024],
                                )
                                for h in range(2):  # 4 transposes/psum tile
                                    pt = ptr.tile([P, 512], bf16, tag="tr")
                                    for j in range(4):
                                        nc.tensor.transpose(
                                            pt[:, j * P : (j + 1) * P],
                                            x_nat[:, (h * 4 + j) * P :
                                                  (h * 4 + j + 1) * P],
                                            ident,
                                        )
                                    dst = xT[:, q * 8 + h * 4 :
                                             q * 8 + h * 4 + 4,
                                             b * P : (b + 1) * P]
                                    src = pt[:].rearrange(
                                        "p (j c) -> p j c", c=P)
                                    if (q * 8 + b * 2 + h) % 2 == 0:
                                        nc.vector.tensor_copy(dst, src)
                                    else:
                                        nc.scalar.copy(dst, src)
                                # layer-1 partials: contract this quarter's
                                # 8 K-chunks for the 256-wide batch pair as
                                # soon as both chunks are transposed; keeps
                                # real (HAM-visible) matmuls flowing.
                                if b % 2 == 1:
                                    bp = b // 2           # batch pair 0..3
                                    off = (bp % 2) * 256  # region in bank
                                    cs = slice((b - 1) * P, (b + 1) * P)
                                    for m in range(2):
                                        mw = P if m == 0 else H1 - P
                                        ps = ps_l1[m][bp // 2]
                                        for k in range(q * 8, q * 8 + 8):
                                            nc.tensor.matmul(
                                                ps[0:mw, off : off + 256],
                                                m1[:, k,
                                                   m * P : m * P + mw],
                                                xT[:, k, cs],
                                                start=False,
                                                stop=(k == NK1 - 1),
                                                skip_group_check=True,
                                            )
                                        if q == 3:
                                            nc.scalar.activation(
                                                h1T[0:mw, m, cs],
                                                ps[0:mw, off : off + 256],
                                                AF.Relu,
                                                bias=b1_sb[0:mw, m : m + 1],
                                            )
                        # mid-chain weight loads: emitted after the x loads
                        nc.gpsimd.memset(w3_sb[:], 0.0)
                        nc.gpsimd.dma_start(w3_sb[0:H2, :], W3_d[:])
                        nc.gpsimd.dma_start(
                            w4_sb[:],
                            W4_d[:].rearrange("(ko p) m -> p ko m", p=P),
                        )
                        nc.gpsimd.memset(wd1_sb[:], 0.0)
                        nc.gpsimd.dma_start(wd1_sb[0:LAT, :], Wd1_d[:])
                        nc.gpsimd.dma_start(
                            wd2_sb[:],
                            Wd2_d[:].rearrange("(ko p) m -> p ko m", p=P),
                        )

                # ------------- layers 2-6 (transposed chain) -------------
                with (
                    tc.tile_pool(name="acts2", bufs=1) as a2pool,
                    tc.tile_pool(name="psum_mm", bufs=6,
                                 space="PSUM") as pmm,
                ):
                    h3T = a2pool.tile([P, D4 // P, B], bf16)
                    zT = a2pool.tile([P, B], bf16)
                    nc.gpsimd.memset(zT[:], 0.0)
                    d1T = a2pool.tile([P, DD1 // P, B], bf16)
                    d2T = a2pool.tile([P, DD2 // P, B], bf16)

                    for n in range(B // NT):
                        ns = slice(n * NT, (n + 1) * NT)
                        # L2: K = 196 (2 padded chunks), M = 10
                        ps = pmm.tile([P, NT], f32, tag="mm")
                        for k in range(2):
                            nc.tensor.matmul(ps[0:H2, :], m2[:, k, :],
                                             h1T[:, k, ns],
                                             start=(k == 0), stop=(k == 1))
                        nc.scalar.activation(h2T[0:H2, ns], ps[0:H2, :],
                                             AF.Relu, bias=b2_sb[:, 0:1])
                        # L3: K = 10 (padded to 128), M = 1024
                        for m in range(D4 // P):
                            ps = pmm.tile([P, NT], f32, tag="mm")
                            nc.tensor.matmul(ps[:],
                                             w3_sb[:, m * P : (m + 1) * P],
                                             h2T[:, ns], start=True,
                                             stop=True)
                            nc.scalar.activation(h3T[:, m, ns], ps[:],
                                                 AF.Relu,
                                                 bias=b3_sb[:, m : m + 1])
                        # L4: K = 1024, M = 32
                        ps = pmm.tile([P, NT], f32, tag="mm")
                        for k in range(D4 // P):
                            nc.tensor.matmul(ps[0:LAT, :], w4_sb[:, k, :],
                                             h3T[:, k, ns], start=(k == 0),
                                             stop=(k == D4 // P - 1))
                        nc.scalar.activation(zT[0:LAT, ns], ps[0:LAT, :],
                                             AF.Relu, bias=b4_sb[:, 0:1])
                        # L5: K = 32 (padded to 128), M = 1024
                        for m in range(DD1 // P):
                            ps = pmm.tile([P, NT], f32, tag="mm")
                            nc.tensor.matmul(ps[:],
                                             wd1_sb[:, m * P : (m + 1) * P],
                                             zT[:, ns], start=True, stop=True)
                            nc.scalar.activation(d1T[:, m, ns], ps[:],
                                                 AF.Relu,
                                                 bias=bd1_sb[:, m : m + 1])
                        # L6: K = 1024, M = 2048
                        for m in range(DD2 // P):
                            ps = pmm.tile([P, NT], f32, tag="mm")
                            for k in range(DD1 // P):
                                nc.tensor.matmul(
                                    ps[:],
                                    wd2_sb[:, k, m * P : (m + 1) * P],
                                    d1T[:, k, ns], start=(k == 0),
                                    stop=(k == DD1 // P - 1),
                                )
                            nc.scalar.activation(d2T[:, m, ns], ps[:],
                                                 AF.Relu,
                                                 bias=bd2_sb[:, m : m + 1])

                    # ---------- layer 7 (flipped, natural output) ---------
                    wd3_r = Wd3_d[:].rearrange("(ko p) n -> p ko n", p=P)
                    with tc.tile_pool(name="wd3", bufs=3) as wpool3:
                        for nn in range(S // NT):
                            wt = wpool3.tile([P, NK7, NT], bf16, tag="wd3")
                            nc.gpsimd.dma_start(
                                wt[:], wd3_r[:, :, nn * NT : (nn + 1) * NT]
                            )
                            for m in range(NB):
                                ps = pmm.tile([P, NT], f32, tag="mm")
                                # bias first: K=1 ones-row outer product
                                # broadcasts bd3 to all 128 batch partitions
                                nc.tensor.matmul(
                                    ps[:], ones_row[0:1, :],
                                    bd3_row[0:1, nn * NT : (nn + 1) * NT],
                                    start=True, stop=False,
                                )
                                for k in range(NK7):
                                    nc.tensor.matmul(
                                        ps[:],
                                        d2T[:, k, m * P : (m + 1) * P],
                                        wt[:, k, :], start=False,
                                        stop=(k == NK7 - 1),
                                    )
                                ot = opool.tile([P, NT], f32, tag="out")
                                nc.scalar.activation(ot[:], ps[:], AF.Sigmoid)
                                nc.sync.dma_start(
                                    out_d[m * P : (m + 1) * P,
                                          nn * NT : (nn + 1) * NT],
                                    ot[:],
                                )

    nc.compile()
    return nc


def _get_nc():
    if "nc" not in _NC_CACHE:
        _NC_CACHE["nc"] = build_nc()
    return _NC_CACHE["nc"]


def kernel(**inputs):
    from concourse.bass_utils import run_bass_kernel_spmd

    nc = _get_nc()
    full = {k: np.ascontiguousarray(np.asarray(v, dtype=np.float32))
            for k, v in inputs.items()}
    x = full.pop("x")
    in_maps = []
    for c in range(N_CORES):
        m = dict(full)
        m["x"] = np.ascontiguousarray(x[c * B : (c + 1) * B])
        in_maps.append(m)
    res = run_bass_kernel_spmd(nc, in_maps, core_ids=list(range(N_CORES)),
                               trace=TRACE)
    _NC_CACHE["last_res"] = res
    out = np.concatenate([res.results[c]["out"] for c in range(N_CORES)],
                         axis=0)
    return out


# revision 15
# speedup vs baseline: 1.0978x; 1.0639x over previous
"""Trainium2 Bass kernel for the CustomAutoencoder problem.

7-layer MLP autoencoder over x[8192, 4096], data-parallel over the batch
axis across 8 NeuronCores (1024 rows/core), weights replicated.

Per-core dataflow (activations kept transposed: features on partitions,
batch on the free axis), bf16 matmul operands with fp32 PSUM accumulation:

  x[1024,4096] --cast-DMA+PE-transpose--> xT
  L1: h1T = relu(M1.T @ xT + b1)      M1 = W1*C1   [196,  B]
  L2: h2T = relu(M2.T @ h1T + b2)     M2 = W2*C2   [10,   B]
  L3: h3T = relu(W3.T @ h2T + b3)                  [1024, B]
  L4: zT  = relu(W4.T @ h3T + b4)                  [32,   B]
  L5: d1T = relu(Wd1.T @ zT + bd1)                 [1024, B]
  L6: d2T = relu(Wd2.T @ d1T + bd2)                [2048, B]
  L7: out = sigmoid(d2 @ Wd3 + bd3)   (operands flipped: d2T chunks are
      lhsT, Wd3 is the moving operand -> natural [B, 4096] output, no
      exit transpose; bd3 enters PSUM via a K=1 ones-row matmul)
"""

import sys

if "/opt/trn_rl_repo" not in sys.path:
    sys.path.insert(0, "/opt/trn_rl_repo")

import numpy as np

B_FULL, S, H1, H2, D4, LAT, DD1, DD2 = 8192, 4096, 196, 10, 1024, 32, 1024, 2048
N_CORES = 8
B = B_FULL // N_CORES          # 1024 rows per core
P = 128                        # partitions
NB = B // P                    # 8 batch chunks per core
NT = 512                       # matmul free-dim tile (one PSUM bank of fp32)

_NC_CACHE = {}
TRACE = False  # set by test.py to capture an NTFF profile of the run


def build_nc():
    import concourse.bacc as bacc
    import concourse.mybir as mybir
    import concourse.tile as tile
    from concourse.masks import make_identity

    f32 = mybir.dt.float32
    bf16 = mybir.dt.bfloat16
    AF = mybir.ActivationFunctionType

    nc = bacc.Bacc("TRN2", target_bir_lowering=False, debug=False,
                   num_devices=N_CORES)

    # ---- DRAM I/O (names match setup_inputs keys; x/out are per-core shards)
    x_d = nc.dram_tensor("x", [B, S], f32, kind="ExternalInput")
    C1_d = nc.dram_tensor("C1", [S, H1], f32, kind="ExternalInput")
    W1_d = nc.dram_tensor("W1", [S, H1], f32, kind="ExternalInput")
    b1_d = nc.dram_tensor("b1", [H1], f32, kind="ExternalInput")
    C2_d = nc.dram_tensor("C2", [H1, H2], f32, kind="ExternalInput")
    W2_d = nc.dram_tensor("W2", [H1, H2], f32, kind="ExternalInput")
    b2_d = nc.dram_tensor("b2", [H2], f32, kind="ExternalInput")
    W3_d = nc.dram_tensor("W3", [H2, D4], f32, kind="ExternalInput")
    b3_d = nc.dram_tensor("b3", [D4], f32, kind="ExternalInput")
    W4_d = nc.dram_tensor("W4", [D4, LAT], f32, kind="ExternalInput")
    b4_d = nc.dram_tensor("b4", [LAT], f32, kind="ExternalInput")
    Wd1_d = nc.dram_tensor("Wd1", [LAT, DD1], f32, kind="ExternalInput")
    bd1_d = nc.dram_tensor("bd1", [DD1], f32, kind="ExternalInput")
    Wd2_d = nc.dram_tensor("Wd2", [DD1, DD2], f32, kind="ExternalInput")
    bd2_d = nc.dram_tensor("bd2", [DD2], f32, kind="ExternalInput")
    Wd3_d = nc.dram_tensor("Wd3", [DD2, S], f32, kind="ExternalInput")
    bd3_d = nc.dram_tensor("bd3", [S], f32, kind="ExternalInput")
    out_d = nc.dram_tensor("out", [B, S], f32, kind="ExternalOutput")

    NK1 = S // P   # 32 K-chunks for layer 1
    NK7 = DD2 // P  # 16 K-chunks for layer 7

    with tile.TileContext(nc) as tc:
        with (
            tc.tile_pool(name="const", bufs=1) as cpool,
            tc.tile_pool(name="acts01", bufs=1) as a0pool,
            tc.tile_pool(name="outp", bufs=4) as opool,
        ):
            # ---------------- constants / small weights ----------------
            ident = cpool.tile([P, P], bf16)
            make_identity(nc, ident)
            ones_row = cpool.tile([1, P], bf16)
            nc.gpsimd.memset(ones_row[:], 1.0)

            # biases as [P, chunks] fp32 (feature index = chunk*128 + p).
            # A direct DMA into that layout degenerates to one 4-byte
            # descriptor per element and clogs the HWDGE queue for tens of
            # microseconds, so load natural [chunks, 128] rows (one fat
            # descriptor per row) and PE-transpose on chip instead.
            ident_f32 = cpool.tile([P, P], f32)
            make_identity(nc, ident_f32)
            b2_sb = cpool.tile([H2, 1], f32)
            nc.sync.dma_start(b2_sb[:, 0:1], b2_d[:])
            b4_sb = cpool.tile([LAT, 1], f32)
            nc.sync.dma_start(b4_sb[:, 0:1], b4_d[:])
            bd3_row = cpool.tile([1, S], bf16)
            nc.gpsimd.dma_start(bd3_row[0:1, :], bd3_d[:])

            # masked layer-2 weights: M2 = W2*C2, K padded to 2x128 with zeros
            m2 = cpool.tile([P, 2, H2], bf16)
            nc.gpsimd.memset(m2[:], 0.0)
            w2_t = cpool.tile([P, 2, H2], bf16)
            nc.gpsimd.memset(w2_t[:], 0.0)
            nc.gpsimd.dma_start(w2_t[:, 0, :], W2_d[0:P, :])
            nc.gpsimd.dma_start(w2_t[0 : H1 - P, 1, :], W2_d[P:H1, :])
            c2_t = cpool.tile([P, 2, H2], bf16)
            nc.gpsimd.memset(c2_t[:], 0.0)
            nc.gpsimd.dma_start(c2_t[:, 0, :], C2_d[0:P, :])
            nc.gpsimd.dma_start(c2_t[0 : H1 - P, 1, :], C2_d[P:H1, :])
            nc.vector.tensor_mul(m2[:], w2_t[:], c2_t[:])

            # small persistent activations
            h1T = a0pool.tile([P, 2, B], bf16)
            nc.gpsimd.memset(h1T[:], 0.0)  # K-pad rows of chunk 1 stay zero
            h2T = a0pool.tile([P, B], bf16)
            nc.gpsimd.memset(h2T[:], 0.0)

            # mid-chain weights: pool opened early (space is reserved), but
            # the DMAs are emitted after the x loads so the x stream wins
            # the SWDGE queue.
            with tc.tile_pool(name="wts2", bufs=1) as wpool2:
                w3_sb = wpool2.tile([P, D4], bf16)
                w4_sb = wpool2.tile([P, D4 // P, LAT], bf16)
                wd1_sb = wpool2.tile([P, DD1], bf16)
                wd2_sb = wpool2.tile([P, DD1 // P, DD2], bf16)

                # ---------------- stage 1: x transpose + layer 1 ---------
                with (
                    tc.tile_pool(name="stage1", bufs=1) as spool,
                    tc.tile_pool(name="psum_tr", bufs=2,
                                 space="PSUM") as ptr,
                    tc.tile_pool(name="psum_l1", bufs=1,
                                 space="PSUM") as pl1,
                ):
                    # PE warm-up: ~40 back-to-back matmuls lift the HAM
                    # clock gate (1.2 -> 2.4 GHz) before the real work.
                    warm_ps = ptr.tile([P, P], f32, tag="warm", bufs=1)
                    for _ in range(40):
                        nc.tensor.matmul(warm_ps[:], ident[:], ident[:],
                                         start=True, stop=True,
                                         skip_group_check=True)

                    # per-partition bias layouts via on-chip transpose
                    def load_bias_t(dst, src_d, nrows, tail):
                        nat = spool.tile([P, P], f32, tag="bias_nat",
                                         bufs=2)
                        if tail:
                            nc.gpsimd.memset(nat[0:nrows, :], 0.0)
                            nc.sync.dma_start(
                                nat[0 : nrows - 1, :], src_d[0 : (nrows - 1) * P])
                            nc.sync.dma_start(
                                nat[nrows - 1 : nrows, 0:tail],
                                src_d[(nrows - 1) * P :])
                        else:
                            nc.sync.dma_start(
                                nat[0:nrows, :],
                                src_d[:].rearrange("(o p) -> o p", p=P))
                        pb = ptr.tile([P, 16], f32, tag="btr", bufs=1)
                        nc.tensor.transpose(pb[:, 0:nrows], nat[0:nrows, :],
                                            ident_f32[0:nrows, 0:nrows])
                        nc.vector.tensor_copy(dst[:], pb[:, 0:nrows])

                    b1_sb = cpool.tile([P, 2], f32)
                    load_bias_t(b1_sb, b1_d, 2, H1 - P)
                    b3_sb = cpool.tile([P, D4 // P], f32)
                    load_bias_t(b3_sb, b3_d, D4 // P, 0)
                    bd1_sb = cpool.tile([P, DD1 // P], f32)
                    load_bias_t(bd1_sb, bd1_d, DD1 // P, 0)
                    bd2_sb = cpool.tile([P, DD2 // P], f32)
                    load_bias_t(bd2_sb, bd2_d, DD2 // P, 0)

                    m1 = spool.tile([P, NK1, H1], bf16)

                    # 4 PSUM accumulators, each holding two 256-wide batch
                    # regions. start=False throughout (a start=True would
                    # clear has_written for the whole bank, wiping the
                    # sibling region), so zero them explicitly first.
                    ps_l1 = [[pl1.tile([P, NT], f32, name=f"l1_{m}_{j}")
                              for j in range(2)] for m in range(2)]
                    for m in range(2):
                        for j in range(2):
                            nc.any.memset(ps_l1[m][j][:], 0.0)

                    w1_r = W1_d[:].rearrange("(ko p) m -> p ko m", p=P)
                    c1_r = C1_d[:].rearrange("(ko p) m -> p ko m", p=P)

                    with tc.tile_pool(name="xbuf", bufs=1) as xpool:
                        xT = xpool.tile([P, NK1, B], bf16)
                        for q in range(4):      # quarters of the 4096 row
                            # this quarter's slice of M1 = W1*C1 (bf16
                            # cast-DMA on the same SWDGE queue, just ahead
                            # of the quarter's x loads)
                            ks = slice(q * 8, (q + 1) * 8)
                            w1s = spool.tile([P, 8, H1], bf16, tag="w1s",
                                             bufs=2)
                            nc.gpsimd.dma_start(w1s[:], w1_r[:, ks, :])
                            c1s = spool.tile([P, 8, H1], bf16, tag="c1s",
                                             bufs=2)
                            nc.gpsimd.dma_start(c1s[:], c1_r[:, ks, :])
                            nc.vector.tensor_mul(m1[:, ks, :], w1s[:],
                                                 c1s[:])
                            for b in range(NB):  # batch chunks of 128
                                x_nat = xpool.tile([P, 1024], bf16,
                                                   tag="xnat", bufs=4)
                                nc.gpsimd.dma_start(
                                    x_nat[:],
                                    x_d[b * P : (b + 1) * P,
                                        q * 1024 : (q + 1) * 1024],
                                )
                                for h in range(2):  # 4 transposes/psum tile
                                    pt = ptr.tile([P, 512], bf16, tag="tr")
                                    for j in range(4):
                                        nc.tensor.transpose(
                                            pt[:, j * P : (j + 1) * P],
                                            x_nat[:, (h * 4 + j) * P :
                                                  (h * 4 + j + 1) * P],
                                            ident,
                                        )
                                    dst = xT[:, q * 8 + h * 4 :
                                             q * 8 + h * 4 + 4,
                                             b * P : (b + 1) * P]
                                    src = pt[:].rearrange(
                                        "p (j c) -> p j c", c=P)
                                    if (q * 8 + b * 2 + h) % 2 == 0:
                                        nc.vector.tensor_copy(dst, src)
                                    else:
                                        nc.scalar.copy(dst, src)
                                # layer-1 partials: contract this quarter's
                                # 8 K-chunks for the 256-wide batch pair as
                                # soon as both chunks are transposed; keeps
                                # real (HAM-visible) matmuls flowing.
                                if b % 2 == 1:
                                    bp = b // 2           # batch pair 0..3
                                    off = (bp % 2) * 256  # region in bank
                                    cs = slice((b - 1) * P, (b + 1) * P)
                                    for m in range(2):
                                        mw = P if m == 0 else H1 - P
                                        ps = ps_l1[m][bp // 2]
                                        for k in range(q * 8, q * 8 + 8):
                                            nc.tensor.matmul(
                                                ps[0:mw, off : off + 256],
                                                m1[:, k,
                                                   m * P : m * P + mw],
                                                xT[:, k, cs],
                                                start=False,
                                                stop=(k == NK1 - 1),
                                                skip_group_check=True,
                                            )
                                        if q == 3:
                                            nc.scalar.activation(
                                                h1T[0:mw, m, cs],
                                                ps[0:mw, off : off + 256],
                                                AF.Relu,
                                                bias=b1_sb[0:mw, m : m + 1],
                                            )
                        # mid-chain weight loads: emitted after the x loads
                        nc.gpsimd.memset(w3_sb[:], 0.0)
                        nc.gpsimd.dma_start(w3_sb[0:H2, :], W3_d[:])
                        nc.gpsimd.dma_start(
                            w4_sb[:],
                            W4_d[:].rearrange("(ko p) m -> p ko m", p=P),
                        )
                        nc.gpsimd.memset(wd1_sb[:], 0.0)
                        nc.gpsimd.dma_start(wd1_sb[0:LAT, :], Wd1_d[:])
                        nc.gpsimd.dma_start(
                            wd2_sb[:],
                            Wd2_d[:].rearrange("(ko p) m -> p ko m", p=P),
                        )

                # ------------- layers 2-6 (transposed chain) -------------
                with (
                    tc.tile_pool(name="acts2", bufs=1) as a2pool,
                    tc.tile_pool(name="psum_mm", bufs=6,
                                 space="PSUM") as pmm,
                ):
                    h3T = a2pool.tile([P, D4 // P, B], bf16)
                    zT = a2pool.tile([P, B], bf16)
                    nc.gpsimd.memset(zT[:], 0.0)
                    d1T = a2pool.tile([P, DD1 // P, B], bf16)
                    d2T = a2pool.tile([P, DD2 // P, B], bf16)

                    for n in range(B // NT):
                        ns = slice(n * NT, (n + 1) * NT)
                        # L2: K = 196 (2 padded chunks), M = 10
                        ps = pmm.tile([P, NT], f32, tag="mm")
                        for k in range(2):
                            nc.tensor.matmul(ps[0:H2, :], m2[:, k, :],
                                             h1T[:, k, ns],
                                             start=(k == 0), stop=(k == 1))
                        nc.scalar.activation(h2T[0:H2, ns], ps[0:H2, :],
                                             AF.Relu, bias=b2_sb[:, 0:1])
                        # L3: K = 10 (padded to 128), M = 1024
                        for m in range(D4 // P):
                            ps = pmm.tile([P, NT], f32, tag="mm")
                            nc.tensor.matmul(ps[:],
                                             w3_sb[:, m * P : (m + 1) * P],
                                             h2T[:, ns], start=True,
                                             stop=True)
                            nc.scalar.activation(h3T[:, m, ns], ps[:],
                                                 AF.Relu,
                                                 bias=b3_sb[:, m : m + 1])
                        # L4: K = 1024, M = 32
                        ps = pmm.tile([P, NT], f32, tag="mm")
                        for k in range(D4 // P):
                            nc.tensor.matmul(ps[0:LAT, :], w4_sb[:, k, :],
                                             h3T[:, k, ns], start=(k == 0),
                                             stop=(k == D4 // P - 1))
                        nc.scalar.activation(zT[0:LAT, ns], ps[0:LAT, :],
                                             AF.Relu, bias=b4_sb[:, 0:1])
                        # L5: K = 32 (padded to 128), M = 1024
                        for m in range(DD1 // P):
                            ps = pmm.tile([P, NT], f32, tag="mm")
                            nc.tensor.matmul(ps[:],
                                             wd1_sb[:, m * P : (m + 1) * P],
                                             zT[:, ns], start=True, stop=True)
                            nc.scalar.activation(d1T[:, m, ns], ps[:],
                                                 AF.Relu,
                                                 bias=bd1_sb[:, m : m + 1])
                        # L6: K = 1024, M = 2048
                        for m in range(DD2 // P):
                            ps = pmm.tile([P, NT], f32, tag="mm")
                            for k in range(DD1 // P):
                                nc.tensor.matmul(
                                    ps[:],
                                    wd2_sb[:, k, m * P : (m + 1) * P],
                                    d1T[:, k, ns], start=(k == 0),
                                    stop=(k == DD1 // P - 1),
                                )
                            nc.scalar.activation(d2T[:, m, ns], ps[:],
                                                 AF.Relu,
                                                 bias=bd2_sb[:, m : m + 1])

                    # ---------- layer 7 (flipped, natural output) ---------
                    wd3_r = Wd3_d[:].rearrange("(ko p) n -> p ko n", p=P)
                    with tc.tile_pool(name="wd3", bufs=3) as wpool3:
                        for nn in range(S // NT):
                            wt = wpool3.tile([P, NK7, NT], bf16, tag="wd3")
                            nc.gpsimd.dma_start(
                                wt[:], wd3_r[:, :, nn * NT : (nn + 1) * NT]
                            )
                            for m in range(NB):
                                ps = pmm.tile([P, NT], f32, tag="mm")
                                # bias first: K=1 ones-row outer product
                                # broadcasts bd3 to all 128 batch partitions
                                nc.tensor.matmul(
                                    ps[:], ones_row[0:1, :],
                                    bd3_row[0:1, nn * NT : (nn + 1) * NT],
                                    start=True, stop=False,
                                )
                                for k in range(NK7):
                                    nc.tensor.matmul(
                                        ps[:],
                                        d2T[:, k, m * P : (m + 1) * P],
                                        wt[:, k, :], start=False,
                                        stop=(k == NK7 - 1),
                                    )
                                ot = opool.tile([P, NT], f32, tag="out")
                                nc.scalar.activation(ot[:], ps[:], AF.Sigmoid)
                                nc.sync.dma_start(
                                    out_d[m * P : (m + 1) * P,
                                          nn * NT : (nn + 1) * NT],
                                    ot[:],
                                )

    nc.compile()
    return nc


def _get_nc():
    if "nc" not in _NC_CACHE:
        _NC_CACHE["nc"] = build_nc()
    return _NC_CACHE["nc"]


def kernel(**inputs):
    from concourse.bass_utils import run_bass_kernel_spmd

    nc = _get_nc()
    full = {k: np.ascontiguousarray(np.asarray(v, dtype=np.float32))
            for k, v in inputs.items()}
    x = full.pop("x")
    in_maps = []
    for c in range(N_CORES):
        m = dict(full)
        m["x"] = np.ascontiguousarray(x[c * B : (c + 1) * B])
        in_maps.append(m)
    res = run_bass_kernel_spmd(nc, in_maps, core_ids=list(range(N_CORES)),
                               trace=TRACE)
    _NC_CACHE["last_res"] = res
    out = np.concatenate([res.results[c]["out"] for c in range(N_CORES)],
                         axis=0)
    return out
